# revision 56
# baseline (speedup 1.0000x reference)
"""Bahdanau additive attention, data-parallel over batch on 8 TRN2 NeuronCores.

Reference computation (per batch b):
    q   = dec[b] @ Wa + ba                      # [U]
    k   = enc[b] @ Wb + bb                      # [T, U]
    s   = tanh(q + k) @ Wv + bv                 # [T]
    a   = softmax(s)                            # [T]
    ctx = a @ enc[b]                            # [ENC]

Device strategy (per core, 8 batches each):
  - enc is shipped twice in bf16: e-major [ENC, T] for the k-projection
    (contraction over ENC lives on partitions) and t-major [T, ENC] for the
    context reduction (contraction over T on partitions).
  - k-projection: Wb chunks stationary, e-major enc streams; psum [128u, T].
  - tanh fused with the (q + ba + bb) per-partition bias on ScalarE.
  - scores: tanh tile [u, 128t] as stationary, Wv as 1-wide moving operand;
    psum [128t, 16] with t = p*16 + c so the attn store is contiguous.
  - softmax: Exp with accum_out gives per-partition partial sums; a ones
    [128,128] stationary matmul broadcasts the total to every partition;
    reciprocal + tensor_scalar_mul normalize.  (max-subtraction skipped:
    |score| <= ||Wv||_1 + |bv| ~ 12, exp stays well inside fp32 range;
    softmax is shift-invariant so bv is dropped entirely.)
  - context: attn column stationary (M=1), t-major enc streams; psum [1, ENC].
"""

import numpy as np
import ml_dtypes

B, T, ENC, DEC, U = 64, 2048, 512, 512, 128
NCORES = 8
BL = B // NCORES  # batches per core
P = 128

_NC_CACHE = {}


def _build_nc():
    import concourse.mybir as mybir
    from concourse import bacc
    from concourse.masks import make_identity
    from concourse.tile import TileContext

    f32 = mybir.dt.float32
    bf16 = mybir.dt.bfloat16
    AF = mybir.ActivationFunctionType

    nc = bacc.Bacc()

    # enc ships host-pretiled to the exact SBUF layout: one contiguous 16KB
    # run per partition per batch (few descriptors, max-size DMA packets)
    enc_e = nc.declare_dram_parameter("enc_e", [BL, P, ENC // P, T], bf16, isOutput=False)
    enc_t = nc.declare_dram_parameter("enc_t", [BL, P, T // P, ENC], bf16, isOutput=False)
    decT = nc.declare_dram_parameter("decT", [DEC, BL], f32, isOutput=False)
    Wa = nc.declare_dram_parameter("Wa", [DEC, U], f32, isOutput=False)
    Wb = nc.declare_dram_parameter("Wb", [ENC, U], bf16, isOutput=False)
    Wv = nc.declare_dram_parameter("Wv", [U, 1], bf16, isOutput=False)
    bias2 = nc.declare_dram_parameter("bias2", [U, 2], f32, isOutput=False)
    ctx_out = nc.declare_dram_parameter("ctx", [BL, ENC], f32, isOutput=True)
    attn_out = nc.declare_dram_parameter("attn", [BL, T], f32, isOutput=True)

    EJ = ENC // P  # 4 e-chunks
    TC = T // P    # 16 t-chunks
    TH = 512       # psum_k chunk size
    NH = T // TH   # 4 chunks

    with TileContext(nc) as tc:
        with (
            tc.tile_pool(name="const", bufs=1) as cpool,
            tc.tile_pool(name="enc", bufs=2) as epool,
            tc.tile_pool(name="work", bufs=2) as wpool,
            tc.tile_pool(name="pk", bufs=2, space="PSUM") as pkpool,
            tc.tile_pool(name="psmall", bufs=1, space="PSUM") as pspool,
        ):
            # ---- constants ----
            wa_sb = cpool.tile([P, DEC // P, U], f32)
            nc.sync.dma_start(wa_sb, Wa[:].rearrange("(j p) u -> p j u", p=P))
            wb_sb = cpool.tile([P, EJ, U], bf16)
            nc.sync.dma_start(wb_sb, Wb[:].rearrange("(j p) u -> p j u", p=P))
            wv_sb = cpool.tile([P, 1], bf16)
            nc.sync.dma_start(wv_sb, Wv[:])
            bias2_sb = cpool.tile([P, 2], f32)
            nc.sync.dma_start(bias2_sb, bias2[:])
            decT_sb = cpool.tile([P, DEC // P, BL], f32)
            nc.sync.dma_start(decT_sb, decT[:].rearrange("(j p) b -> p j b", p=P))
            ones_sb = cpool.tile([P, P], bf16)
            nc.vector.memset(ones_sb, 1.0)
            ident_sb = cpool.tile([P, P], f32)
            make_identity(nc, ident_sb)

            bias_sum = cpool.tile([P, 1], f32)
            nc.vector.tensor_add(
                out=bias_sum, in0=bias2_sb[:, 0:1], in1=bias2_sb[:, 1:2]
            )

            # ---- q^T = Wa^T @ dec^T  -> [U, BL], then + (ba+bb) ----
            psum_q = pspool.tile([P, BL], f32, tag="tiny")
            for j in range(DEC // P):
                nc.tensor.matmul(
                    psum_q,
                    lhsT=wa_sb[:, j],
                    rhs=decT_sb[:, j],
                    start=(j == 0),
                    stop=(j == DEC // P - 1),
                )
            qtot = cpool.tile([P, BL], f32)
            nc.scalar.activation(qtot, psum_q, AF.Identity, bias=bias_sum)

            # ---- per-batch pipeline ----
            for b in range(BL):
                ee = epool.tile([P, EJ, T], bf16, tag="ee")
                nc.sync.dma_start(ee, enc_e[b])
                et = epool.tile([P, TC, ENC], bf16, tag="et")
                nc.scalar.dma_start(et, enc_t[b])

                tanh_sb = wpool.tile([P, T], bf16, tag="tanh")
                psum_s = pspool.tile([P, TC], f32, tag="ps", bufs=2)

                # k^T = Wb^T @ enc^T in psum chunks; tanh(k^T + q + biases);
                # then scores for the t-chunks of that piece:
                #   psum_s[p, c] = score(t = c*128 + p)
                for h in range(NH):
                    psum_k = pkpool.tile([P, TH], f32, tag="pk")
                    for j in range(EJ):
                        nc.tensor.matmul(
                            psum_k,
                            lhsT=wb_sb[:, j],
                            rhs=ee[:, j, h * TH : (h + 1) * TH],
                            start=(j == 0),
                            stop=(j == EJ - 1),
                        )
                    nc.scalar.activation(
                        tanh_sb[:, h * TH : (h + 1) * TH],
                        psum_k,
                        AF.Tanh,
                        bias=qtot[:, b : b + 1],
                    )
                    for c in range(h * (TH // P), (h + 1) * (TH // P)):
                        nc.tensor.matmul(
                            psum_s[:, c : c + 1],
                            lhsT=tanh_sb[:, c * P : (c + 1) * P],
                            rhs=wv_sb,
                            start=True,
                            stop=True,
                        )

                # softmax (no max subtraction; see module docstring)
                exp_sb = wpool.tile([P, TC], f32, tag="exp")
                rowsum = wpool.tile([P, 1], f32, tag="rowsum")
                nc.scalar.activation(exp_sb, psum_s, AF.Exp, accum_out=rowsum)

                rowsum_bf = wpool.tile([P, 1], bf16, tag="rowsum_bf")
                nc.vector.tensor_copy(rowsum_bf, rowsum)
                psum_bc = pspool.tile([P, 1], f32, tag="tiny")
                nc.tensor.matmul(
                    psum_bc, lhsT=ones_sb, rhs=rowsum_bf, start=True, stop=True
                )
                inv_col = wpool.tile([P, 1], f32, tag="inv")
                nc.vector.reciprocal(inv_col, psum_bc)

                attn_f = wpool.tile([P, TC], f32, tag="attn_f")
                nc.vector.tensor_scalar_mul(attn_f, exp_sb, inv_col)
                attn_b = wpool.tile([P, TC], bf16, tag="attn_b")
                nc.vector.tensor_scalar_mul(attn_b, exp_sb, inv_col)

                # transpose attn so the DRAM store is contiguous per partition
                psum_at = pspool.tile([TC, P], f32, tag="pat")
                nc.tensor.transpose(psum_at, attn_f, ident_sb)
                attn_t = wpool.tile([TC, P], f32, tag="attn_t")
                nc.vector.tensor_copy(attn_t, psum_at)
                nc.sync.dma_start(
                    attn_out[b].rearrange("(c p) -> c p", p=P), attn_t
                )

                # context: ctx[e] = sum_c sum_p attn[c*128+p] * et[p,c,e]
                # Two ping-pong psum banks so consecutive accumulating matmuls
                # hit different banks (same-region accumulation serializes the
                # PE fill/drain: measured 630ns vs ~380ns per N=512 matmul).
                psum_cA = pspool.tile([1, ENC], f32, tag="pctxA")
                psum_cB = pspool.tile([1, ENC], f32, tag="pctxB")
                for c in range(TC):
                    nc.tensor.matmul(
                        psum_cA if c % 2 == 0 else psum_cB,
                        lhsT=attn_b[:, c : c + 1],
                        rhs=et[:, c],
                        start=(c < 2),
                        stop=(c >= TC - 2),
                    )
                ctx_sb = wpool.tile([1, ENC], f32, tag="ctx")
                nc.vector.tensor_copy(ctx_sb, psum_cA)
                nc.vector.tensor_add(out=ctx_sb, in0=ctx_sb, in1=psum_cB)
                nc.sync.dma_start(ctx_out[b : b + 1, :], ctx_sb)

    nc.finalize()
    return nc


def build_in_maps(decoder_hidden, encoder_outputs, Wa, ba, Wb, bb, Wv, bv):
    bf = ml_dtypes.bfloat16
    enc = np.asarray(encoder_outputs, dtype=np.float32)
    dec = np.asarray(decoder_hidden, dtype=np.float32)
    wa = np.ascontiguousarray(np.asarray(Wa, dtype=np.float32))
    wb = np.ascontiguousarray(np.asarray(Wb, dtype=np.float32)).astype(bf)
    wv = np.ascontiguousarray(np.asarray(Wv, dtype=np.float32)).astype(bf)
    bias2 = np.ascontiguousarray(
        np.stack([np.asarray(ba, np.float32), np.asarray(bb, np.float32)], axis=1)
    )

    in_maps = []
    for i in range(NCORES):
        sl = slice(i * BL, (i + 1) * BL)
        enc_i = enc[sl].astype(bf)
        # pretile to the SBUF layouts: et[b,p,c,e]=enc[b,c*128+p,e],
        # ee[b,p,j,t]=enc[b,t,j*128+p]
        et_h = np.ascontiguousarray(
            enc_i.reshape(BL, T // P, P, ENC).transpose(0, 2, 1, 3)
        )
        ee_h = np.ascontiguousarray(
            enc_i.transpose(0, 2, 1).reshape(BL, ENC // P, P, T).transpose(0, 2, 1, 3)
        )
        in_maps.append(
            {
                "enc_t": et_h,
                "enc_e": ee_h,
                "decT": np.ascontiguousarray(dec[sl].T),
                "Wa": wa,
                "Wb": wb,
                "Wv": wv,
                "bias2": bias2,
            }
        )
    return in_maps


def kernel(decoder_hidden, encoder_outputs, Wa, ba, Wb, bb, Wv, bv):
    from concourse.bass_utils import run_bass_kernel_spmd

    if "nc" not in _NC_CACHE:
        _NC_CACHE["nc"] = _build_nc()
    nc = _NC_CACHE["nc"]

    in_maps = build_in_maps(
        decoder_hidden, encoder_outputs, Wa, ba, Wb, bb, Wv, bv
    )
    res = run_bass_kernel_spmd(nc, in_maps, core_ids=list(range(NCORES))).results
    ctx = np.concatenate([res[i]["ctx"] for i in range(NCORES)], axis=0)
    attn = np.concatenate([res[i]["attn"] for i in range(NCORES)], axis=0)
    return ctx.astype(np.float32), attn.astype(np.float32)[..., None]


# revision 57
# speedup vs baseline: 1.0356x; 1.0356x over previous
"""Bahdanau additive attention, data-parallel over batch on 8 TRN2 NeuronCores.

Reference computation (per batch b):
    q   = dec[b] @ Wa + ba                      # [U]
    k   = enc[b] @ Wb + bb                      # [T, U]
    s   = tanh(q + k) @ Wv + bv                 # [T]
    a   = softmax(s)                            # [T]
    ctx = a @ enc[b]                            # [ENC]

Device strategy (per core, 8 batches each):
  - enc is shipped twice in bf16: e-major [ENC, T] for the k-projection
    (contraction over ENC lives on partitions) and t-major [T, ENC] for the
    context reduction (contraction over T on partitions).
  - k-projection: Wb chunks stationary, e-major enc streams; psum [128u, T].
  - tanh fused with the (q + ba + bb) per-partition bias on ScalarE.
  - scores: tanh tile [u, 128t] as stationary, Wv as 1-wide moving operand;
    psum [128t, 16] with t = p*16 + c so the attn store is contiguous.
  - softmax: Exp with accum_out gives per-partition partial sums; a ones
    [128,128] stationary matmul broadcasts the total to every partition;
    reciprocal + tensor_scalar_mul normalize.  (max-subtraction skipped:
    |score| <= ||Wv||_1 + |bv| ~ 12, exp stays well inside fp32 range;
    softmax is shift-invariant so bv is dropped entirely.)
  - context: attn column stationary (M=1), t-major enc streams; psum [1, ENC].
"""

import numpy as np
import ml_dtypes

B, T, ENC, DEC, U = 64, 2048, 512, 512, 128
NCORES = 8
BL = B // NCORES  # batches per core
P = 128

_NC_CACHE = {}


def _build_nc():
    import concourse.mybir as mybir
    from concourse import bacc
    from concourse.masks import make_identity
    from concourse.tile import TileContext

    f32 = mybir.dt.float32
    bf16 = mybir.dt.bfloat16
    AF = mybir.ActivationFunctionType

    nc = bacc.Bacc()

    # enc ships host-pretiled to the exact SBUF layout: one contiguous 16KB
    # run per partition per batch (few descriptors, max-size DMA packets)
    enc_e = nc.declare_dram_parameter("enc_e", [BL, P, ENC // P, T], bf16, isOutput=False)
    enc_t = nc.declare_dram_parameter("enc_t", [BL, P, T // P, ENC], bf16, isOutput=False)
    decT = nc.declare_dram_parameter("decT", [DEC, BL], f32, isOutput=False)
    Wa = nc.declare_dram_parameter("Wa", [DEC, U], f32, isOutput=False)
    Wb = nc.declare_dram_parameter("Wb", [ENC, U], bf16, isOutput=False)
    Wv = nc.declare_dram_parameter("Wv", [U, 1], bf16, isOutput=False)
    bias2 = nc.declare_dram_parameter("bias2", [U, 2], f32, isOutput=False)
    ctx_out = nc.declare_dram_parameter("ctx", [BL, ENC], f32, isOutput=True)
    attn_out = nc.declare_dram_parameter("attn", [BL, T], f32, isOutput=True)

    EJ = ENC // P  # 4 e-chunks
    TC = T // P    # 16 t-chunks
    TH = 512       # psum_k chunk size
    NH = T // TH   # 4 chunks

    with TileContext(nc) as tc:
        with (
            tc.tile_pool(name="const", bufs=1) as cpool,
            tc.tile_pool(name="enc", bufs=2) as epool,
            tc.tile_pool(name="work", bufs=2) as wpool,
            tc.tile_pool(name="pk", bufs=2, space="PSUM") as pkpool,
            tc.tile_pool(name="psmall", bufs=1, space="PSUM") as pspool,
        ):
            # ---- constants ----
            wa_sb = cpool.tile([P, DEC // P, U], f32)
            nc.sync.dma_start(wa_sb, Wa[:].rearrange("(j p) u -> p j u", p=P))
            wb_sb = cpool.tile([P, EJ, U], bf16)
            nc.sync.dma_start(wb_sb, Wb[:].rearrange("(j p) u -> p j u", p=P))
            wv_sb = cpool.tile([P, 1], bf16)
            nc.sync.dma_start(wv_sb, Wv[:])
            bias2_sb = cpool.tile([P, 2], f32)
            nc.sync.dma_start(bias2_sb, bias2[:])
            decT_sb = cpool.tile([P, DEC // P, BL], f32)
            nc.sync.dma_start(decT_sb, decT[:].rearrange("(j p) b -> p j b", p=P))
            ones_sb = cpool.tile([P, P], bf16)
            nc.vector.memset(ones_sb, 1.0)
            ident_sb = cpool.tile([P, P], f32)
            make_identity(nc, ident_sb)

            bias_sum = cpool.tile([P, 1], f32)
            nc.vector.tensor_add(
                out=bias_sum, in0=bias2_sb[:, 0:1], in1=bias2_sb[:, 1:2]
            )

            # ---- q^T = Wa^T @ dec^T  -> [U, BL], then + (ba+bb) ----
            psum_q = pspool.tile([P, BL], f32, tag="tiny")
            for j in range(DEC // P):
                nc.tensor.matmul(
                    psum_q,
                    lhsT=wa_sb[:, j],
                    rhs=decT_sb[:, j],
                    start=(j == 0),
                    stop=(j == DEC // P - 1),
                )
            qtot = cpool.tile([P, BL], f32)
            nc.scalar.activation(qtot, psum_q, AF.Identity, bias=bias_sum)

            # ---- per-batch pipeline ----
            # ee[b+1] is issued BEFORE et[b]: the next k-projection gates the
            # PE while et is only needed late (context), so the e-major load
            # should win HBM arbitration over the t-major one.
            ee_tiles = {}
            ee0 = epool.tile([P, EJ, T], bf16, tag="ee", name="ee_first")
            nc.sync.dma_start(ee0, enc_e[0])
            ee_tiles[0] = ee0
            for b in range(BL):
                if b + 1 < BL:
                    ee_n = epool.tile([P, EJ, T], bf16, tag="ee", name=f"ee_{b+1}")
                    nc.sync.dma_start(ee_n, enc_e[b + 1])
                    ee_tiles[b + 1] = ee_n
                ee = ee_tiles.pop(b)
                et = epool.tile([P, TC, ENC], bf16, tag="et")
                nc.scalar.dma_start(et, enc_t[b])

                tanh_sb = wpool.tile([P, T], bf16, tag="tanh")
                psum_s = pspool.tile([P, TC], f32, tag="ps", bufs=2)

                # k^T = Wb^T @ enc^T in psum chunks; tanh(k^T + q + biases);
                # then scores for the t-chunks of that piece:
                #   psum_s[p, c] = score(t = c*128 + p)
                for h in range(NH):
                    psum_k = pkpool.tile([P, TH], f32, tag="pk")
                    for j in range(EJ):
                        nc.tensor.matmul(
                            psum_k,
                            lhsT=wb_sb[:, j],
                            rhs=ee[:, j, h * TH : (h + 1) * TH],
                            start=(j == 0),
                            stop=(j == EJ - 1),
                        )
                    nc.scalar.activation(
                        tanh_sb[:, h * TH : (h + 1) * TH],
                        psum_k,
                        AF.Tanh,
                        bias=qtot[:, b : b + 1],
                    )
                    for c in range(h * (TH // P), (h + 1) * (TH // P)):
                        nc.tensor.matmul(
                            psum_s[:, c : c + 1],
                            lhsT=tanh_sb[:, c * P : (c + 1) * P],
                            rhs=wv_sb,
                            start=True,
                            stop=True,
                        )

                # softmax (no max subtraction; see module docstring)
                exp_sb = wpool.tile([P, TC], f32, tag="exp")
                rowsum = wpool.tile([P, 1], f32, tag="rowsum")
                nc.scalar.activation(exp_sb, psum_s, AF.Exp, accum_out=rowsum)

                rowsum_bf = wpool.tile([P, 1], bf16, tag="rowsum_bf")
                nc.vector.tensor_copy(rowsum_bf, rowsum)
                psum_bc = pspool.tile([P, 1], f32, tag="tiny")
                nc.tensor.matmul(
                    psum_bc, lhsT=ones_sb, rhs=rowsum_bf, start=True, stop=True
                )
                inv_col = wpool.tile([P, 1], f32, tag="inv")
                nc.vector.reciprocal(inv_col, psum_bc)

                attn_f = wpool.tile([P, TC], f32, tag="attn_f")
                nc.vector.tensor_scalar_mul(attn_f, exp_sb, inv_col)
                attn_b = wpool.tile([P, TC], bf16, tag="attn_b")
                nc.vector.tensor_scalar_mul(attn_b, exp_sb, inv_col)

                # transpose attn so the DRAM store is contiguous per partition
                psum_at = pspool.tile([TC, P], f32, tag="pat")
                nc.tensor.transpose(psum_at, attn_f, ident_sb)
                attn_t = wpool.tile([TC, P], f32, tag="attn_t")
                nc.vector.tensor_copy(attn_t, psum_at)
                nc.sync.dma_start(
                    attn_out[b].rearrange("(c p) -> c p", p=P), attn_t
                )

                # context: ctx[e] = sum_c sum_p attn[c*128+p] * et[p,c,e]
                # Two ping-pong psum banks so consecutive accumulating matmuls
                # hit different banks (same-region accumulation serializes the
                # PE fill/drain: measured 630ns vs ~380ns per N=512 matmul).
                psum_cA = pspool.tile([1, ENC], f32, tag="pctxA")
                psum_cB = pspool.tile([1, ENC], f32, tag="pctxB")
                for c in range(TC):
                    nc.tensor.matmul(
                        psum_cA if c % 2 == 0 else psum_cB,
                        lhsT=attn_b[:, c : c + 1],
                        rhs=et[:, c],
                        start=(c < 2),
                        stop=(c >= TC - 2),
                    )
                ctx_sb = wpool.tile([1, ENC], f32, tag="ctx")
                nc.vector.tensor_copy(ctx_sb, psum_cA)
                nc.vector.tensor_add(out=ctx_sb, in0=ctx_sb, in1=psum_cB)
                nc.sync.dma_start(ctx_out[b : b + 1, :], ctx_sb)

    nc.finalize()
    return nc


def build_in_maps(decoder_hidden, encoder_outputs, Wa, ba, Wb, bb, Wv, bv):
    bf = ml_dtypes.bfloat16
    enc = np.asarray(encoder_outputs, dtype=np.float32)
    dec = np.asarray(decoder_hidden, dtype=np.float32)
    wa = np.ascontiguousarray(np.asarray(Wa, dtype=np.float32))
    wb = np.ascontiguousarray(np.asarray(Wb, dtype=np.float32)).astype(bf)
    wv = np.ascontiguousarray(np.asarray(Wv, dtype=np.float32)).astype(bf)
    bias2 = np.ascontiguousarray(
        np.stack([np.asarray(ba, np.float32), np.asarray(bb, np.float32)], axis=1)
    )

    in_maps = []
    for i in range(NCORES):
        sl = slice(i * BL, (i + 1) * BL)
        enc_i = enc[sl].astype(bf)
        # pretile to the SBUF layouts: et[b,p,c,e]=enc[b,c*128+p,e],
        # ee[b,p,j,t]=enc[b,t,j*128+p]
        et_h = np.ascontiguousarray(
            enc_i.reshape(BL, T // P, P, ENC).transpose(0, 2, 1, 3)
        )
        ee_h = np.ascontiguousarray(
            enc_i.transpose(0, 2, 1).reshape(BL, ENC // P, P, T).transpose(0, 2, 1, 3)
        )
        in_maps.append(
            {
                "enc_t": et_h,
                "enc_e": ee_h,
                "decT": np.ascontiguousarray(dec[sl].T),
                "Wa": wa,
                "Wb": wb,
                "Wv": wv,
                "bias2": bias2,
            }
        )
    return in_maps


def kernel(decoder_hidden, encoder_outputs, Wa, ba, Wb, bb, Wv, bv):
    from concourse.bass_utils import run_bass_kernel_spmd

    if "nc" not in _NC_CACHE:
        _NC_CACHE["nc"] = _build_nc()
    nc = _NC_CACHE["nc"]

    in_maps = build_in_maps(
        decoder_hidden, encoder_outputs, Wa, ba, Wb, bb, Wv, bv
    )
    res = run_bass_kernel_spmd(nc, in_maps, core_ids=list(range(NCORES))).results
    ctx = np.concatenate([res[i]["ctx"] for i in range(NCORES)], axis=0)
    attn = np.concatenate([res[i]["attn"] for i in range(NCORES)], axis=0)
    return ctx.astype(np.float32), attn.astype(np.float32)[..., None]


# revision 58
# speedup vs baseline: 1.0806x; 1.0435x over previous
"""Bahdanau additive attention, data-parallel over batch on 8 TRN2 NeuronCores.

Reference computation (per batch b):
    q   = dec[b] @ Wa + ba                      # [U]
    k   = enc[b] @ Wb + bb                      # [T, U]
    s   = tanh(q + k) @ Wv + bv                 # [T]
    a   = softmax(s)                            # [T]
    ctx = a @ enc[b]                            # [ENC]

Device strategy (per core, 8 batches each):
  - enc is shipped twice in bf16: e-major [ENC, T] for the k-projection
    (contraction over ENC lives on partitions) and t-major [T, ENC] for the
    context reduction (contraction over T on partitions).
  - k-projection: Wb chunks stationary, e-major enc streams; psum [128u, T].
  - tanh fused with the (q + ba + bb) per-partition bias on ScalarE.
  - scores: tanh tile [u, 128t] as stationary, Wv as 1-wide moving operand;
    psum [128t, 16] with t = p*16 + c so the attn store is contiguous.
  - softmax: Exp with accum_out gives per-partition partial sums; a ones
    [128,128] stationary matmul broadcasts the total to every partition;
    reciprocal + tensor_scalar_mul normalize.  (max-subtraction skipped:
    |score| <= ||Wv||_1 + |bv| ~ 12, exp stays well inside fp32 range;
    softmax is shift-invariant so bv is dropped entirely.)
  - context: attn column stationary (M=1), t-major enc streams; psum [1, ENC].
"""

import numpy as np
import ml_dtypes

B, T, ENC, DEC, U = 64, 2048, 512, 512, 128
NCORES = 8
BL = B // NCORES  # batches per core
P = 128

_NC_CACHE = {}


def _build_nc():
    import concourse.mybir as mybir
    from concourse import bacc
    from concourse.masks import make_identity
    from concourse.tile import TileContext

    f32 = mybir.dt.float32
    bf16 = mybir.dt.bfloat16
    AF = mybir.ActivationFunctionType

    nc = bacc.Bacc()

    # enc ships host-pretiled to the exact SBUF layout: one contiguous 16KB
    # run per partition per batch (few descriptors, max-size DMA packets)
    enc_e = nc.declare_dram_parameter("enc_e", [BL, P, ENC // P, T], bf16, isOutput=False)
    enc_t = nc.declare_dram_parameter("enc_t", [BL, P, T // P, ENC], bf16, isOutput=False)
    decT = nc.declare_dram_parameter("decT", [DEC, BL], f32, isOutput=False)
    Wa = nc.declare_dram_parameter("Wa", [DEC, U], f32, isOutput=False)
    Wb = nc.declare_dram_parameter("Wb", [ENC, U], bf16, isOutput=False)
    Wv = nc.declare_dram_parameter("Wv", [U, 1], bf16, isOutput=False)
    bias2 = nc.declare_dram_parameter("bias2", [U, 2], f32, isOutput=False)
    ctx_out = nc.declare_dram_parameter("ctx", [BL, ENC], f32, isOutput=True)
    attn_out = nc.declare_dram_parameter("attn", [BL, T], f32, isOutput=True)

    EJ = ENC // P  # 4 e-chunks
    TC = T // P    # 16 t-chunks
    TH = 512       # psum_k chunk size
    NH = T // TH   # 4 chunks

    with TileContext(nc) as tc:
        with (
            tc.tile_pool(name="const", bufs=1) as cpool,
            tc.tile_pool(name="enc", bufs=2) as epool,
            tc.tile_pool(name="work", bufs=2) as wpool,
            tc.tile_pool(name="pk", bufs=2, space="PSUM") as pkpool,
            tc.tile_pool(name="psmall", bufs=1, space="PSUM") as pspool,
        ):
            # ---- constants ----
            wa_sb = cpool.tile([P, DEC // P, U], f32)
            nc.sync.dma_start(wa_sb, Wa[:].rearrange("(j p) u -> p j u", p=P))
            wb_sb = cpool.tile([P, EJ, U], bf16)
            nc.sync.dma_start(wb_sb, Wb[:].rearrange("(j p) u -> p j u", p=P))
            wv_sb = cpool.tile([P, 1], bf16)
            nc.sync.dma_start(wv_sb, Wv[:])
            bias2_sb = cpool.tile([P, 2], f32)
            nc.sync.dma_start(bias2_sb, bias2[:])
            decT_sb = cpool.tile([P, DEC // P, BL], f32)
            nc.sync.dma_start(decT_sb, decT[:].rearrange("(j p) b -> p j b", p=P))
            ones_sb = cpool.tile([P, P], bf16)
            nc.vector.memset(ones_sb, 1.0)
            ident_sb = cpool.tile([P, P], f32)
            make_identity(nc, ident_sb)

            bias_sum = cpool.tile([P, 1], f32)
            nc.vector.tensor_add(
                out=bias_sum, in0=bias2_sb[:, 0:1], in1=bias2_sb[:, 1:2]
            )

            # ---- q^T = Wa^T @ dec^T  -> [U, BL], then + (ba+bb) ----
            psum_q = pspool.tile([P, BL], f32, tag="tiny")
            for j in range(DEC // P):
                nc.tensor.matmul(
                    psum_q,
                    lhsT=wa_sb[:, j],
                    rhs=decT_sb[:, j],
                    start=(j == 0),
                    stop=(j == DEC // P - 1),
                )
            qtot = cpool.tile([P, BL], f32)
            nc.scalar.activation(qtot, psum_q, AF.Identity, bias=bias_sum)

            # ---- per-batch pipeline ----
            for b in range(BL):
                ee = epool.tile([P, EJ, T], bf16, tag="ee")
                nc.sync.dma_start(ee, enc_e[b])
                et = epool.tile([P, TC, ENC], bf16, tag="et")
                nc.scalar.dma_start(et, enc_t[b])

                tanh_sb = wpool.tile([P, T], bf16, tag="tanh")
                psum_s = pspool.tile([P, TC], f32, tag="ps", bufs=2)

                # k^T = Wb^T @ enc^T in psum chunks; tanh(k^T + q + biases);
                # then scores for the t-chunks of that piece:
                #   psum_s[p, c] = score(t = c*128 + p)
                for h in range(NH):
                    psum_k = pkpool.tile([P, TH], f32, tag="pk")
                    for j in range(EJ):
                        nc.tensor.matmul(
                            psum_k,
                            lhsT=wb_sb[:, j],
                            rhs=ee[:, j, h * TH : (h + 1) * TH],
                            start=(j == 0),
                            stop=(j == EJ - 1),
                        )
                    nc.scalar.activation(
                        tanh_sb[:, h * TH : (h + 1) * TH],
                        psum_k,
                        AF.Tanh,
                        bias=qtot[:, b : b + 1],
                    )
                    for c in range(h * (TH // P), (h + 1) * (TH // P)):
                        nc.tensor.matmul(
                            psum_s[:, c : c + 1],
                            lhsT=tanh_sb[:, c * P : (c + 1) * P],
                            rhs=wv_sb,
                            start=True,
                            stop=True,
                        )

                # softmax (no max subtraction; see module docstring)
                exp_sb = wpool.tile([P, TC], f32, tag="exp")
                rowsum = wpool.tile([P, 1], f32, tag="rowsum")
                nc.scalar.activation(exp_sb, psum_s, AF.Exp, accum_out=rowsum)

                rowsum_bf = wpool.tile([P, 1], bf16, tag="rowsum_bf")
                nc.vector.tensor_copy(rowsum_bf, rowsum)
                psum_bc = pspool.tile([P, 1], f32, tag="tiny")
                nc.tensor.matmul(
                    psum_bc, lhsT=ones_sb, rhs=rowsum_bf, start=True, stop=True
                )
                inv_col = wpool.tile([P, 1], f32, tag="inv")
                nc.vector.reciprocal(inv_col, psum_bc)

                attn_f = wpool.tile([P, TC], f32, tag="attn_f")
                nc.vector.tensor_scalar_mul(attn_f, exp_sb, inv_col)
                attn_b = wpool.tile([P, TC], bf16, tag="attn_b")
                nc.vector.tensor_scalar_mul(attn_b, exp_sb, inv_col)

                # transpose attn so the DRAM store is contiguous per partition
                psum_at = pspool.tile([TC, P], f32, tag="pat")
                nc.tensor.transpose(psum_at, attn_f, ident_sb)
                attn_t = wpool.tile([TC, P], f32, tag="attn_t")
                nc.vector.tensor_copy(attn_t, psum_at)
                nc.sync.dma_start(
                    attn_out[b].rearrange("(c p) -> c p", p=P), attn_t
                )

                # context: ctx[e] = sum_c sum_p attn[c*128+p] * et[p,c,e]
                # Two ping-pong psum banks so consecutive accumulating matmuls
                # hit different banks (same-region accumulation serializes the
                # PE fill/drain: measured 630ns vs ~380ns per N=512 matmul).
                psum_cA = pspool.tile([1, ENC], f32, tag="pctxA")
                psum_cB = pspool.tile([1, ENC], f32, tag="pctxB")
                for c in range(TC):
                    nc.tensor.matmul(
                        psum_cA if c % 2 == 0 else psum_cB,
                        lhsT=attn_b[:, c : c + 1],
                        rhs=et[:, c],
                        start=(c < 2),
                        stop=(c >= TC - 2),
                    )
                ctx_sb = wpool.tile([1, ENC], f32, tag="ctx")
                nc.vector.tensor_copy(ctx_sb, psum_cA)
                nc.vector.tensor_add(out=ctx_sb, in0=ctx_sb, in1=psum_cB)
                nc.sync.dma_start(ctx_out[b : b + 1, :], ctx_sb)

    nc.finalize()
    return nc


def build_in_maps(decoder_hidden, encoder_outputs, Wa, ba, Wb, bb, Wv, bv):
    bf = ml_dtypes.bfloat16
    enc = np.asarray(encoder_outputs, dtype=np.float32)
    dec = np.asarray(decoder_hidden, dtype=np.float32)
    wa = np.ascontiguousarray(np.asarray(Wa, dtype=np.float32))
    wb = np.ascontiguousarray(np.asarray(Wb, dtype=np.float32)).astype(bf)
    wv = np.ascontiguousarray(np.asarray(Wv, dtype=np.float32)).astype(bf)
    bias2 = np.ascontiguousarray(
        np.stack([np.asarray(ba, np.float32), np.asarray(bb, np.float32)], axis=1)
    )

    in_maps = []
    for i in range(NCORES):
        sl = slice(i * BL, (i + 1) * BL)
        enc_i = enc[sl].astype(bf)
        # pretile to the SBUF layouts: et[b,p,c,e]=enc[b,c*128+p,e],
        # ee[b,p,j,t]=enc[b,t,j*128+p]
        et_h = np.ascontiguousarray(
            enc_i.reshape(BL, T // P, P, ENC).transpose(0, 2, 1, 3)
        )
        ee_h = np.ascontiguousarray(
            enc_i.transpose(0, 2, 1).reshape(BL, ENC // P, P, T).transpose(0, 2, 1, 3)
        )
        in_maps.append(
            {
                "enc_t": et_h,
                "enc_e": ee_h,
                "decT": np.ascontiguousarray(dec[sl].T),
                "Wa": wa,
                "Wb": wb,
                "Wv": wv,
                "bias2": bias2,
            }
        )
    return in_maps


def kernel(decoder_hidden, encoder_outputs, Wa, ba, Wb, bb, Wv, bv):
    from concourse.bass_utils import run_bass_kernel_spmd

    if "nc" not in _NC_CACHE:
        _NC_CACHE["nc"] = _build_nc()
    nc = _NC_CACHE["nc"]

    in_maps = build_in_maps(
        decoder_hidden, encoder_outputs, Wa, ba, Wb, bb, Wv, bv
    )
    res = run_bass_kernel_spmd(nc, in_maps, core_ids=list(range(NCORES))).results
    ctx = np.concatenate([res[i]["ctx"] for i in range(NCORES)], axis=0)
    attn = np.concatenate([res[i]["attn"] for i in range(NCORES)], axis=0)
    return ctx.astype(np.float32), attn.astype(np.float32)[..., None]


# revision 64
# speedup vs baseline: 1.1346x; 1.0500x over previous
"""Bahdanau additive attention, data-parallel over batch on 8 TRN2 NeuronCores.

Reference computation (per batch b):
    q   = dec[b] @ Wa + ba                      # [U]
    k   = enc[b] @ Wb + bb                      # [T, U]
    s   = tanh(q + k) @ Wv + bv                 # [T]
    a   = softmax(s)                            # [T]
    ctx = a @ enc[b]                            # [ENC]

Device strategy (per core, 8 batches each):
  - enc is shipped twice in bf16: e-major [ENC, T] for the k-projection
    (contraction over ENC lives on partitions) and t-major [T, ENC] for the
    context reduction (contraction over T on partitions).
  - k-projection: Wb chunks stationary, e-major enc streams; psum [128u, T].
  - tanh fused with the (q + ba + bb) per-partition bias on ScalarE.
  - scores: tanh tile [u, 128t] as stationary, Wv as 1-wide moving operand;
    psum [128t, 16] with t = p*16 + c so the attn store is contiguous.
  - softmax: Exp with accum_out gives per-partition partial sums; a ones
    [128,128] stationary matmul broadcasts the total to every partition;
    reciprocal + tensor_scalar_mul normalize.  (max-subtraction skipped:
    |score| <= ||Wv||_1 + |bv| ~ 12, exp stays well inside fp32 range;
    softmax is shift-invariant so bv is dropped entirely.)
  - context: attn column stationary (M=1), t-major enc streams; psum [1, ENC].
"""

import numpy as np
import ml_dtypes

B, T, ENC, DEC, U = 64, 2048, 512, 512, 128
NCORES = 8
BL = B // NCORES  # batches per core
P = 128

_NC_CACHE = {}


def _build_nc():
    import concourse.mybir as mybir
    from concourse import bacc
    from concourse.masks import make_identity
    from concourse.tile import TileContext

    f32 = mybir.dt.float32
    bf16 = mybir.dt.bfloat16
    AF = mybir.ActivationFunctionType

    nc = bacc.Bacc()

    # enc ships host-pretiled to the exact SBUF layout, in T-halves: each
    # half is one contiguous 8KB run per partition (full-rate packets) and a
    # separate tile, so kproj/ctx start after half a load instead of a full one
    enc_e = nc.declare_dram_parameter(
        "enc_e", [BL, 2, P, ENC // P, T // 2], bf16, isOutput=False
    )
    enc_t = nc.declare_dram_parameter(
        "enc_t", [BL, 2, P, T // P // 2, ENC], bf16, isOutput=False
    )
    decT = nc.declare_dram_parameter("decT", [DEC, BL], f32, isOutput=False)
    Wa = nc.declare_dram_parameter("Wa", [DEC, U], f32, isOutput=False)
    Wb = nc.declare_dram_parameter("Wb", [ENC, U], bf16, isOutput=False)
    Wv = nc.declare_dram_parameter("Wv", [U, 1], bf16, isOutput=False)
    bias2 = nc.declare_dram_parameter("bias2", [U, 2], f32, isOutput=False)
    ctx_out = nc.declare_dram_parameter("ctx", [BL, ENC], f32, isOutput=True)
    attn_out = nc.declare_dram_parameter("attn", [BL, T], f32, isOutput=True)

    EJ = ENC // P  # 4 e-chunks
    TC = T // P    # 16 t-chunks
    TH = 512       # psum_k chunk size
    NH = T // TH   # 4 chunks

    with TileContext(nc) as tc:
        with (
            tc.tile_pool(name="const", bufs=1) as cpool,
            tc.tile_pool(name="enc", bufs=2) as epool,
            tc.tile_pool(name="work", bufs=2) as wpool,
            tc.tile_pool(name="pk", bufs=2, space="PSUM") as pkpool,
            tc.tile_pool(name="psmall", bufs=1, space="PSUM") as pspool,
        ):
            # ---- constants ----
            wa_sb = cpool.tile([P, DEC // P, U], f32)
            nc.sync.dma_start(wa_sb, Wa[:].rearrange("(j p) u -> p j u", p=P))
            wb_sb = cpool.tile([P, EJ, U], bf16)
            nc.sync.dma_start(wb_sb, Wb[:].rearrange("(j p) u -> p j u", p=P))
            wv_sb = cpool.tile([P, 1], bf16)
            nc.sync.dma_start(wv_sb, Wv[:])
            bias2_sb = cpool.tile([P, 2], f32)
            nc.sync.dma_start(bias2_sb, bias2[:])
            decT_sb = cpool.tile([P, DEC // P, BL], f32)
            nc.sync.dma_start(decT_sb, decT[:].rearrange("(j p) b -> p j b", p=P))
            ones_sb = cpool.tile([P, P], bf16)
            nc.vector.memset(ones_sb, 1.0)
            ident_sb = cpool.tile([P, P], f32)
            make_identity(nc, ident_sb)

            bias_sum = cpool.tile([P, 1], f32)
            nc.vector.tensor_add(
                out=bias_sum, in0=bias2_sb[:, 0:1], in1=bias2_sb[:, 1:2]
            )

            # ---- q^T = Wa^T @ dec^T  -> [U, BL], then + (ba+bb) ----
            psum_q = pspool.tile([P, BL], f32, tag="tiny")
            for j in range(DEC // P):
                nc.tensor.matmul(
                    psum_q,
                    lhsT=wa_sb[:, j],
                    rhs=decT_sb[:, j],
                    start=(j == 0),
                    stop=(j == DEC // P - 1),
                )
            qtot = cpool.tile([P, BL], f32)
            nc.scalar.activation(qtot, psum_q, AF.Identity, bias=bias_sum)

            # ---- per-batch pipeline ----
            for b in range(BL):
                ee_h = []
                et_h = []
                for x in range(2):
                    eh = epool.tile([P, EJ, T // 2], bf16, tag=f"ee{x}",
                                    name=f"ee{x}_{b}")
                    nc.sync.dma_start(eh, enc_e[b, x])
                    ee_h.append(eh)
                    th_ = epool.tile([P, TC // 2, ENC], bf16, tag=f"et{x}",
                                     name=f"et{x}_{b}")
                    nc.scalar.dma_start(th_, enc_t[b, x])
                    et_h.append(th_)

                tanh_sb = wpool.tile([P, T], bf16, tag="tanh")
                psum_s = pspool.tile([P, TC], f32, tag="ps", bufs=2)

                # k^T = Wb^T @ enc^T in psum chunks; tanh(k^T + q + biases);
                # then scores for the t-chunks of that piece:
                #   psum_s[p, c] = score(t = c*128 + p)
                for h in range(NH):
                    psum_k = pkpool.tile([P, TH], f32, tag="pk")
                    for j in range(EJ):
                        nc.tensor.matmul(
                            psum_k,
                            lhsT=wb_sb[:, j],
                            rhs=ee_h[h // 2][
                                :, j, (h % 2) * TH : (h % 2 + 1) * TH
                            ],
                            start=(j == 0),
                            stop=(j == EJ - 1),
                        )
                    nc.scalar.activation(
                        tanh_sb[:, h * TH : (h + 1) * TH],
                        psum_k,
                        AF.Tanh,
                        bias=qtot[:, b : b + 1],
                    )
                    for c in range(h * (TH // P), (h + 1) * (TH // P)):
                        nc.tensor.matmul(
                            psum_s[:, c : c + 1],
                            lhsT=tanh_sb[:, c * P : (c + 1) * P],
                            rhs=wv_sb,
                            start=True,
                            stop=True,
                        )

                # softmax (no max subtraction; see module docstring)
                exp_sb = wpool.tile([P, TC], f32, tag="exp")
                rowsum = wpool.tile([P, 1], f32, tag="rowsum")
                nc.scalar.activation(exp_sb, psum_s, AF.Exp, accum_out=rowsum)

                rowsum_bf = wpool.tile([P, 1], bf16, tag="rowsum_bf")
                nc.vector.tensor_copy(rowsum_bf, rowsum)
                psum_bc = pspool.tile([P, 1], f32, tag="tiny")
                nc.tensor.matmul(
                    psum_bc, lhsT=ones_sb, rhs=rowsum_bf, start=True, stop=True
                )
                inv_col = wpool.tile([P, 1], f32, tag="inv")
                nc.vector.reciprocal(inv_col, psum_bc)

                attn_f = wpool.tile([P, TC], f32, tag="attn_f")
                nc.vector.tensor_scalar_mul(attn_f, exp_sb, inv_col)
                attn_b = wpool.tile([P, TC], bf16, tag="attn_b")
                nc.vector.tensor_scalar_mul(attn_b, exp_sb, inv_col)

                # transpose attn so the DRAM store is contiguous per partition
                psum_at = pspool.tile([TC, P], f32, tag="pat")
                nc.tensor.transpose(psum_at, attn_f, ident_sb)
                attn_t = wpool.tile([TC, P], f32, tag="attn_t")
                nc.vector.tensor_copy(attn_t, psum_at)
                nc.sync.dma_start(
                    attn_out[b].rearrange("(c p) -> c p", p=P), attn_t
                )

                # context: ctx[e] = sum_c sum_p attn[c*128+p] * et[p,c,e]
                # Two ping-pong psum banks so consecutive accumulating matmuls
                # hit different banks (same-region accumulation serializes the
                # PE fill/drain: measured 630ns vs ~380ns per N=512 matmul).
                psum_cA = pspool.tile([1, ENC], f32, tag="pctxA")
                psum_cB = pspool.tile([1, ENC], f32, tag="pctxB")
                for c in range(TC):
                    nc.tensor.matmul(
                        psum_cA if c % 2 == 0 else psum_cB,
                        lhsT=attn_b[:, c : c + 1],
                        rhs=et_h[c // (TC // 2)][:, c % (TC // 2)],
                        start=(c < 2),
                        stop=(c >= TC - 2),
                    )
                ctx_sb = wpool.tile([1, ENC], f32, tag="ctx")
                nc.vector.tensor_copy(ctx_sb, psum_cA)
                nc.vector.tensor_add(out=ctx_sb, in0=ctx_sb, in1=psum_cB)
                nc.sync.dma_start(ctx_out[b : b + 1, :], ctx_sb)

    nc.finalize()
    return nc


def build_in_maps(decoder_hidden, encoder_outputs, Wa, ba, Wb, bb, Wv, bv):
    bf = ml_dtypes.bfloat16
    enc = np.asarray(encoder_outputs, dtype=np.float32)
    dec = np.asarray(decoder_hidden, dtype=np.float32)
    wa = np.ascontiguousarray(np.asarray(Wa, dtype=np.float32))
    wb = np.ascontiguousarray(np.asarray(Wb, dtype=np.float32)).astype(bf)
    wv = np.ascontiguousarray(np.asarray(Wv, dtype=np.float32)).astype(bf)
    bias2 = np.ascontiguousarray(
        np.stack([np.asarray(ba, np.float32), np.asarray(bb, np.float32)], axis=1)
    )

    in_maps = []
    for i in range(NCORES):
        sl = slice(i * BL, (i + 1) * BL)
        enc_i = enc[sl].astype(bf)
        # pretile to the SBUF layouts, T-halves outermost:
        # et[b,ch,p,c',e] = enc[b,(ch*8+c')*128+p,e]
        # ee[b,th,p,j,t'] = enc[b,th*1024+t',j*128+p]
        et_h = np.ascontiguousarray(
            enc_i.reshape(BL, 2, T // P // 2, P, ENC).transpose(0, 1, 3, 2, 4)
        )
        ee_h = np.ascontiguousarray(
            enc_i.transpose(0, 2, 1)
            .reshape(BL, ENC // P, P, 2, T // 2)
            .transpose(0, 3, 2, 1, 4)
        )
        in_maps.append(
            {
                "enc_t": et_h,
                "enc_e": ee_h,
                "decT": np.ascontiguousarray(dec[sl].T),
                "Wa": wa,
                "Wb": wb,
                "Wv": wv,
                "bias2": bias2,
            }
        )
    return in_maps


def kernel(decoder_hidden, encoder_outputs, Wa, ba, Wb, bb, Wv, bv):
    from concourse.bass_utils import run_bass_kernel_spmd

    if "nc" not in _NC_CACHE:
        _NC_CACHE["nc"] = _build_nc()
    nc = _NC_CACHE["nc"]

    in_maps = build_in_maps(
        decoder_hidden, encoder_outputs, Wa, ba, Wb, bb, Wv, bv
    )
    res = run_bass_kernel_spmd(nc, in_maps, core_ids=list(range(NCORES))).results
    ctx = np.concatenate([res[i]["ctx"] for i in range(NCORES)], axis=0)
    attn = np.concatenate([res[i]["attn"] for i in range(NCORES)], axis=0)
    return ctx.astype(np.float32), attn.astype(np.float32)[..., None]


# revision 65
# speedup vs baseline: 1.2003x; 1.0579x over previous
"""Bahdanau additive attention, data-parallel over batch on 8 TRN2 NeuronCores.

Reference computation (per batch b):
    q   = dec[b] @ Wa + ba                      # [U]
    k   = enc[b] @ Wb + bb                      # [T, U]
    s   = tanh(q + k) @ Wv + bv                 # [T]
    a   = softmax(s)                            # [T]
    ctx = a @ enc[b]                            # [ENC]

Device strategy (per core, 8 batches each):
  - enc is shipped twice in bf16: e-major [ENC, T] for the k-projection
    (contraction over ENC lives on partitions) and t-major [T, ENC] for the
    context reduction (contraction over T on partitions).
  - k-projection: Wb chunks stationary, e-major enc streams; psum [128u, T].
  - tanh fused with the (q + ba + bb) per-partition bias on ScalarE.
  - scores: tanh tile [u, 128t] as stationary, Wv as 1-wide moving operand;
    psum [128t, 16] with t = p*16 + c so the attn store is contiguous.
  - softmax: Exp with accum_out gives per-partition partial sums; a ones
    [128,128] stationary matmul broadcasts the total to every partition;
    reciprocal + tensor_scalar_mul normalize.  (max-subtraction skipped:
    |score| <= ||Wv||_1 + |bv| ~ 12, exp stays well inside fp32 range;
    softmax is shift-invariant so bv is dropped entirely.)
  - context: attn column stationary (M=1), t-major enc streams; psum [1, ENC].
"""

import numpy as np
import ml_dtypes

B, T, ENC, DEC, U = 64, 2048, 512, 512, 128
NCORES = 8
BL = B // NCORES  # batches per core
P = 128

_NC_CACHE = {}


def _build_nc():
    import concourse.mybir as mybir
    from concourse import bacc
    from concourse.masks import make_identity
    from concourse.tile import TileContext

    f32 = mybir.dt.float32
    bf16 = mybir.dt.bfloat16
    AF = mybir.ActivationFunctionType

    nc = bacc.Bacc()

    # enc ships host-pretiled to the exact SBUF layout, in T-halves: each
    # half is one contiguous 8KB run per partition (full-rate packets) and a
    # separate tile, so kproj/ctx start after half a load instead of a full one
    enc_e = nc.declare_dram_parameter(
        "enc_e", [BL, 4, P, ENC // P, T // 4], bf16, isOutput=False
    )
    enc_t = nc.declare_dram_parameter(
        "enc_t", [BL, 4, P, T // P // 4, ENC], bf16, isOutput=False
    )
    decT = nc.declare_dram_parameter("decT", [DEC, BL], f32, isOutput=False)
    Wa = nc.declare_dram_parameter("Wa", [DEC, U], f32, isOutput=False)
    Wb = nc.declare_dram_parameter("Wb", [ENC, U], bf16, isOutput=False)
    Wv = nc.declare_dram_parameter("Wv", [U, 1], bf16, isOutput=False)
    bias2 = nc.declare_dram_parameter("bias2", [U, 2], f32, isOutput=False)
    ctx_out = nc.declare_dram_parameter("ctx", [BL, ENC], f32, isOutput=True)
    attn_out = nc.declare_dram_parameter("attn", [BL, T], f32, isOutput=True)

    EJ = ENC // P  # 4 e-chunks
    TC = T // P    # 16 t-chunks
    TH = 512       # psum_k chunk size
    NH = T // TH   # 4 chunks

    with TileContext(nc) as tc:
        with (
            tc.tile_pool(name="const", bufs=1) as cpool,
            tc.tile_pool(name="enc", bufs=2) as epool,
            tc.tile_pool(name="work", bufs=2) as wpool,
            tc.tile_pool(name="pk", bufs=2, space="PSUM") as pkpool,
            tc.tile_pool(name="psmall", bufs=1, space="PSUM") as pspool,
        ):
            # ---- constants ----
            wa_sb = cpool.tile([P, DEC // P, U], f32)
            nc.sync.dma_start(wa_sb, Wa[:].rearrange("(j p) u -> p j u", p=P))
            wb_sb = cpool.tile([P, EJ, U], bf16)
            nc.sync.dma_start(wb_sb, Wb[:].rearrange("(j p) u -> p j u", p=P))
            wv_sb = cpool.tile([P, 1], bf16)
            nc.sync.dma_start(wv_sb, Wv[:])
            bias2_sb = cpool.tile([P, 2], f32)
            nc.sync.dma_start(bias2_sb, bias2[:])
            decT_sb = cpool.tile([P, DEC // P, BL], f32)
            nc.sync.dma_start(decT_sb, decT[:].rearrange("(j p) b -> p j b", p=P))
            ones_sb = cpool.tile([P, P], bf16)
            nc.vector.memset(ones_sb, 1.0)
            ident_sb = cpool.tile([P, P], f32)
            make_identity(nc, ident_sb)

            bias_sum = cpool.tile([P, 1], f32)
            nc.vector.tensor_add(
                out=bias_sum, in0=bias2_sb[:, 0:1], in1=bias2_sb[:, 1:2]
            )

            # ---- q^T = Wa^T @ dec^T  -> [U, BL], then + (ba+bb) ----
            psum_q = pspool.tile([P, BL], f32, tag="tiny")
            for j in range(DEC // P):
                nc.tensor.matmul(
                    psum_q,
                    lhsT=wa_sb[:, j],
                    rhs=decT_sb[:, j],
                    start=(j == 0),
                    stop=(j == DEC // P - 1),
                )
            qtot = cpool.tile([P, BL], f32)
            nc.scalar.activation(qtot, psum_q, AF.Identity, bias=bias_sum)

            # ---- per-batch pipeline ----
            for b in range(BL):
                ee_h = []
                et_h = []
                for x in range(4):
                    eh = epool.tile([P, EJ, T // 4], bf16, tag=f"ee{x}",
                                    name=f"ee{x}_{b}")
                    nc.sync.dma_start(eh, enc_e[b, x])
                    ee_h.append(eh)
                    th_ = epool.tile([P, TC // 4, ENC], bf16, tag=f"et{x}",
                                     name=f"et{x}_{b}")
                    nc.scalar.dma_start(th_, enc_t[b, x])
                    et_h.append(th_)

                tanh_sb = wpool.tile([P, T], bf16, tag="tanh")
                psum_s = pspool.tile([P, TC], f32, tag="ps", bufs=2)

                # k^T = Wb^T @ enc^T in psum chunks; tanh(k^T + q + biases);
                # then scores for the t-chunks of that piece:
                #   psum_s[p, c] = score(t = c*128 + p)
                for h in range(NH):
                    psum_k = pkpool.tile([P, TH], f32, tag="pk")
                    for j in range(EJ):
                        nc.tensor.matmul(
                            psum_k,
                            lhsT=wb_sb[:, j],
                            rhs=ee_h[h][:, j],
                            start=(j == 0),
                            stop=(j == EJ - 1),
                        )
                    nc.scalar.activation(
                        tanh_sb[:, h * TH : (h + 1) * TH],
                        psum_k,
                        AF.Tanh,
                        bias=qtot[:, b : b + 1],
                    )
                    for c in range(h * (TH // P), (h + 1) * (TH // P)):
                        nc.tensor.matmul(
                            psum_s[:, c : c + 1],
                            lhsT=tanh_sb[:, c * P : (c + 1) * P],
                            rhs=wv_sb,
                            start=True,
                            stop=True,
                        )

                # softmax (no max subtraction; see module docstring)
                exp_sb = wpool.tile([P, TC], f32, tag="exp")
                rowsum = wpool.tile([P, 1], f32, tag="rowsum")
                nc.scalar.activation(exp_sb, psum_s, AF.Exp, accum_out=rowsum)

                rowsum_bf = wpool.tile([P, 1], bf16, tag="rowsum_bf")
                nc.vector.tensor_copy(rowsum_bf, rowsum)
                psum_bc = pspool.tile([P, 1], f32, tag="tiny")
                nc.tensor.matmul(
                    psum_bc, lhsT=ones_sb, rhs=rowsum_bf, start=True, stop=True
                )
                inv_col = wpool.tile([P, 1], f32, tag="inv")
                nc.vector.reciprocal(inv_col, psum_bc)

                attn_f = wpool.tile([P, TC], f32, tag="attn_f")
                nc.vector.tensor_scalar_mul(attn_f, exp_sb, inv_col)
                attn_b = wpool.tile([P, TC], bf16, tag="attn_b")
                nc.vector.tensor_scalar_mul(attn_b, exp_sb, inv_col)

                # transpose attn so the DRAM store is contiguous per partition
                psum_at = pspool.tile([TC, P], f32, tag="pat")
                nc.tensor.transpose(psum_at, attn_f, ident_sb)
                attn_t = wpool.tile([TC, P], f32, tag="attn_t")
                nc.vector.tensor_copy(attn_t, psum_at)
                nc.sync.dma_start(
                    attn_out[b].rearrange("(c p) -> c p", p=P), attn_t
                )

                # context: ctx[e] = sum_c sum_p attn[c*128+p] * et[p,c,e]
                # Two ping-pong psum banks so consecutive accumulating matmuls
                # hit different banks (same-region accumulation serializes the
                # PE fill/drain: measured 630ns vs ~380ns per N=512 matmul).
                psum_cA = pspool.tile([1, ENC], f32, tag="pctxA")
                psum_cB = pspool.tile([1, ENC], f32, tag="pctxB")
                for c in range(TC):
                    nc.tensor.matmul(
                        psum_cA if c % 2 == 0 else psum_cB,
                        lhsT=attn_b[:, c : c + 1],
                        rhs=et_h[c // (TC // 4)][:, c % (TC // 4)],
                        start=(c < 2),
                        stop=(c >= TC - 2),
                    )
                ctx_sb = wpool.tile([1, ENC], f32, tag="ctx")
                nc.vector.tensor_copy(ctx_sb, psum_cA)
                nc.vector.tensor_add(out=ctx_sb, in0=ctx_sb, in1=psum_cB)
                nc.sync.dma_start(ctx_out[b : b + 1, :], ctx_sb)

    nc.finalize()
    return nc


def build_in_maps(decoder_hidden, encoder_outputs, Wa, ba, Wb, bb, Wv, bv):
    bf = ml_dtypes.bfloat16
    enc = np.asarray(encoder_outputs, dtype=np.float32)
    dec = np.asarray(decoder_hidden, dtype=np.float32)
    wa = np.ascontiguousarray(np.asarray(Wa, dtype=np.float32))
    wb = np.ascontiguousarray(np.asarray(Wb, dtype=np.float32)).astype(bf)
    wv = np.ascontiguousarray(np.asarray(Wv, dtype=np.float32)).astype(bf)
    bias2 = np.ascontiguousarray(
        np.stack([np.asarray(ba, np.float32), np.asarray(bb, np.float32)], axis=1)
    )

    in_maps = []
    for i in range(NCORES):
        sl = slice(i * BL, (i + 1) * BL)
        enc_i = enc[sl].astype(bf)
        # pretile to the SBUF layouts, T-halves outermost:
        # et[b,ch,p,c',e] = enc[b,(ch*8+c')*128+p,e]
        # ee[b,th,p,j,t'] = enc[b,th*1024+t',j*128+p]
        et_h = np.ascontiguousarray(
            enc_i.reshape(BL, 4, T // P // 4, P, ENC).transpose(0, 1, 3, 2, 4)
        )
        ee_h = np.ascontiguousarray(
            enc_i.transpose(0, 2, 1)
            .reshape(BL, ENC // P, P, 4, T // 4)
            .transpose(0, 3, 2, 1, 4)
        )
        in_maps.append(
            {
                "enc_t": et_h,
                "enc_e": ee_h,
                "decT": np.ascontiguousarray(dec[sl].T),
                "Wa": wa,
                "Wb": wb,
                "Wv": wv,
                "bias2": bias2,
            }
        )
    return in_maps


def kernel(decoder_hidden, encoder_outputs, Wa, ba, Wb, bb, Wv, bv):
    from concourse.bass_utils import run_bass_kernel_spmd

    if "nc" not in _NC_CACHE:
        _NC_CACHE["nc"] = _build_nc()
    nc = _NC_CACHE["nc"]

    in_maps = build_in_maps(
        decoder_hidden, encoder_outputs, Wa, ba, Wb, bb, Wv, bv
    )
    res = run_bass_kernel_spmd(nc, in_maps, core_ids=list(range(NCORES))).results
    ctx = np.concatenate([res[i]["ctx"] for i in range(NCORES)], axis=0)
    attn = np.concatenate([res[i]["attn"] for i in range(NCORES)], axis=0)
    return ctx.astype(np.float32), attn.astype(np.float32)[..., None]


# revision 67
# speedup vs baseline: 1.2391x; 1.0323x over previous
"""Bahdanau additive attention, data-parallel over batch on 8 TRN2 NeuronCores.

Reference computation (per batch b):
    q   = dec[b] @ Wa + ba                      # [U]
    k   = enc[b] @ Wb + bb                      # [T, U]
    s   = tanh(q + k) @ Wv + bv                 # [T]
    a   = softmax(s)                            # [T]
    ctx = a @ enc[b]                            # [ENC]

Device strategy (per core, 8 batches each):
  - enc is shipped twice in bf16: e-major [ENC, T] for the k-projection
    (contraction over ENC lives on partitions) and t-major [T, ENC] for the
    context reduction (contraction over T on partitions).
  - k-projection: Wb chunks stationary, e-major enc streams; psum [128u, T].
  - tanh fused with the (q + ba + bb) per-partition bias on ScalarE.
  - scores: tanh tile [u, 128t] as stationary, Wv as 1-wide moving operand;
    psum [128t, 16] with t = p*16 + c so the attn store is contiguous.
  - softmax: Exp with accum_out gives per-partition partial sums; a ones
    [128,128] stationary matmul broadcasts the total to every partition;
    reciprocal + tensor_scalar_mul normalize.  (max-subtraction skipped:
    |score| <= ||Wv||_1 + |bv| ~ 12, exp stays well inside fp32 range;
    softmax is shift-invariant so bv is dropped entirely.)
  - context: attn column stationary (M=1), t-major enc streams; psum [1, ENC].
"""

import numpy as np
import ml_dtypes

B, T, ENC, DEC, U = 64, 2048, 512, 512, 128
NCORES = 8
BL = B // NCORES  # batches per core
P = 128

_NC_CACHE = {}


def _build_nc():
    import concourse.mybir as mybir
    from concourse import bacc
    from concourse.masks import make_identity
    from concourse.tile import TileContext

    f32 = mybir.dt.float32
    bf16 = mybir.dt.bfloat16
    AF = mybir.ActivationFunctionType

    nc = bacc.Bacc()

    # enc ships host-pretiled to the exact SBUF layout, in T-halves: each
    # half is one contiguous 8KB run per partition (full-rate packets) and a
    # separate tile, so kproj/ctx start after half a load instead of a full one
    enc_e = nc.declare_dram_parameter(
        "enc_e", [BL, 2, P, ENC // P, T // 2], bf16, isOutput=False
    )
    enc_t = nc.declare_dram_parameter(
        "enc_t", [BL, 2, P, T // P // 2, ENC], bf16, isOutput=False
    )
    decT = nc.declare_dram_parameter("decT", [DEC, BL], f32, isOutput=False)
    Wa = nc.declare_dram_parameter("Wa", [DEC, U], f32, isOutput=False)
    Wb = nc.declare_dram_parameter("Wb", [ENC, U], bf16, isOutput=False)
    Wv = nc.declare_dram_parameter("Wv", [U, 1], bf16, isOutput=False)
    bias2 = nc.declare_dram_parameter("bias2", [U, 2], f32, isOutput=False)
    ctx_out = nc.declare_dram_parameter("ctx", [BL, ENC], f32, isOutput=True)
    attn_out = nc.declare_dram_parameter("attn", [BL, T], f32, isOutput=True)

    EJ = ENC // P  # 4 e-chunks
    TC = T // P    # 16 t-chunks
    TH = 512       # psum_k chunk size
    NH = T // TH   # 4 chunks

    with TileContext(nc) as tc:
        with (
            tc.tile_pool(name="const", bufs=1) as cpool,
            tc.tile_pool(name="enc", bufs=2) as epool,
            tc.tile_pool(name="work", bufs=2) as wpool,
            tc.tile_pool(name="pk", bufs=2, space="PSUM") as pkpool,
            tc.tile_pool(name="psmall", bufs=1, space="PSUM") as pspool,
        ):
            # ---- constants ----
            wa_sb = cpool.tile([P, DEC // P, U], f32)
            nc.sync.dma_start(wa_sb, Wa[:].rearrange("(j p) u -> p j u", p=P))
            wb_sb = cpool.tile([P, EJ, U], bf16)
            nc.sync.dma_start(wb_sb, Wb[:].rearrange("(j p) u -> p j u", p=P))
            wv_sb = cpool.tile([P, 1], bf16)
            nc.sync.dma_start(wv_sb, Wv[:])
            bias2_sb = cpool.tile([P, 2], f32)
            nc.sync.dma_start(bias2_sb, bias2[:])
            decT_sb = cpool.tile([P, DEC // P, BL], f32)
            nc.sync.dma_start(decT_sb, decT[:].rearrange("(j p) b -> p j b", p=P))
            ones_sb = cpool.tile([P, P], bf16)
            nc.vector.memset(ones_sb, 1.0)
            ident_sb = cpool.tile([P, P], f32)
            make_identity(nc, ident_sb)

            bias_sum = cpool.tile([P, 1], f32)
            nc.vector.tensor_add(
                out=bias_sum, in0=bias2_sb[:, 0:1], in1=bias2_sb[:, 1:2]
            )

            # ---- q^T = Wa^T @ dec^T  -> [U, BL], then + (ba+bb) ----
            psum_q = pspool.tile([P, BL], f32, tag="tiny")
            for j in range(DEC // P):
                nc.tensor.matmul(
                    psum_q,
                    lhsT=wa_sb[:, j],
                    rhs=decT_sb[:, j],
                    start=(j == 0),
                    stop=(j == DEC // P - 1),
                )
            qtot = cpool.tile([P, BL], f32)
            nc.scalar.activation(qtot, psum_q, AF.Identity, bias=bias_sum)

            # ---- per-batch pipeline ----
            for b in range(BL):
                ee_h = []
                et_h = []
                for x in range(2):
                    eh = epool.tile([P, EJ, T // 2], bf16, tag=f"ee{x}",
                                    name=f"ee{x}_{b}")
                    nc.sync.dma_start(eh, enc_e[b, x])
                    ee_h.append(eh)
                    th_ = epool.tile([P, TC // 2, ENC], bf16, tag=f"et{x}",
                                     name=f"et{x}_{b}")
                    nc.scalar.dma_start(th_, enc_t[b, x])
                    et_h.append(th_)

                tanh_sb = wpool.tile([P, T], bf16, tag="tanh")
                psum_s = pspool.tile([P, TC], f32, tag="ps", bufs=2)

                # k^T = Wb^T @ enc^T in psum chunks; tanh(k^T + q + biases);
                # then scores for the t-chunks of that piece:
                #   psum_s[p, c] = score(t = c*128 + p)
                for h in range(NH):
                    psum_k = pkpool.tile([P, TH], f32, tag="pk")
                    for j in range(EJ):
                        nc.tensor.matmul(
                            psum_k,
                            lhsT=wb_sb[:, j],
                            rhs=ee_h[h // 2][
                                :, j, (h % 2) * TH : (h % 2 + 1) * TH
                            ],
                            start=(j == 0),
                            stop=(j == EJ - 1),
                        )
                    nc.scalar.activation(
                        tanh_sb[:, h * TH : (h + 1) * TH],
                        psum_k,
                        AF.Tanh,
                        bias=qtot[:, b : b + 1],
                    )
                    for c in range(h * (TH // P), (h + 1) * (TH // P)):
                        nc.tensor.matmul(
                            psum_s[:, c : c + 1],
                            lhsT=tanh_sb[:, c * P : (c + 1) * P],
                            rhs=wv_sb,
                            start=True,
                            stop=True,
                        )

                # softmax (no max subtraction; see module docstring)
                exp_sb = wpool.tile([P, TC], f32, tag="exp")
                rowsum = wpool.tile([P, 1], f32, tag="rowsum")
                nc.scalar.activation(exp_sb, psum_s, AF.Exp, accum_out=rowsum)

                exp_b = wpool.tile([P, TC], bf16, tag="exp_b")
                nc.vector.tensor_copy(exp_b, exp_sb)

                rowsum_bf = wpool.tile([P, 1], bf16, tag="rowsum_bf")
                nc.vector.tensor_copy(rowsum_bf, rowsum)
                psum_bc = pspool.tile([P, 1], f32, tag="tiny")
                nc.tensor.matmul(
                    psum_bc, lhsT=ones_sb, rhs=rowsum_bf, start=True, stop=True
                )
                inv_col = wpool.tile([P, 1], f32, tag="inv")
                nc.vector.reciprocal(inv_col, psum_bc)

                attn_f = wpool.tile([P, TC], f32, tag="attn_f")
                nc.vector.tensor_scalar_mul(attn_f, exp_sb, inv_col)

                # transpose attn so the DRAM store is contiguous per partition
                psum_at = pspool.tile([TC, P], f32, tag="pat")
                nc.tensor.transpose(psum_at, attn_f, ident_sb)
                attn_t = wpool.tile([TC, P], f32, tag="attn_t")
                nc.vector.tensor_copy(attn_t, psum_at)
                nc.sync.dma_start(
                    attn_out[b].rearrange("(c p) -> c p", p=P), attn_t
                )

                # context: ctx[e] = sum_c sum_p attn[c*128+p] * et[p,c,e]
                # Two ping-pong psum banks so consecutive accumulating matmuls
                # hit different banks (same-region accumulation serializes the
                # PE fill/drain: measured 630ns vs ~380ns per N=512 matmul).
                psum_cA = pspool.tile([1, ENC], f32, tag="pctxA")
                psum_cB = pspool.tile([1, ENC], f32, tag="pctxB")
                for c in range(TC):
                    nc.tensor.matmul(
                        psum_cA if c % 2 == 0 else psum_cB,
                        lhsT=exp_b[:, c : c + 1],
                        rhs=et_h[c // (TC // 2)][:, c % (TC // 2)],
                        start=(c < 2),
                        stop=(c >= TC - 2),
                    )
                ctx_sb = wpool.tile([1, ENC], f32, tag="ctx")
                nc.vector.tensor_copy(ctx_sb, psum_cA)
                nc.vector.tensor_add(out=ctx_sb, in0=ctx_sb, in1=psum_cB)
                nc.vector.tensor_scalar_mul(ctx_sb, ctx_sb, inv_col[0:1, :1])
                nc.sync.dma_start(ctx_out[b : b + 1, :], ctx_sb)

    nc.finalize()
    return nc


def build_in_maps(decoder_hidden, encoder_outputs, Wa, ba, Wb, bb, Wv, bv):
    bf = ml_dtypes.bfloat16
    enc = np.asarray(encoder_outputs, dtype=np.float32)
    dec = np.asarray(decoder_hidden, dtype=np.float32)
    wa = np.ascontiguousarray(np.asarray(Wa, dtype=np.float32))
    wb = np.ascontiguousarray(np.asarray(Wb, dtype=np.float32)).astype(bf)
    wv = np.ascontiguousarray(np.asarray(Wv, dtype=np.float32)).astype(bf)
    bias2 = np.ascontiguousarray(
        np.stack([np.asarray(ba, np.float32), np.asarray(bb, np.float32)], axis=1)
    )

    in_maps = []
    for i in range(NCORES):
        sl = slice(i * BL, (i + 1) * BL)
        enc_i = enc[sl].astype(bf)
        # pretile to the SBUF layouts, T-halves outermost:
        # et[b,ch,p,c',e] = enc[b,(ch*8+c')*128+p,e]
        # ee[b,th,p,j,t'] = enc[b,th*1024+t',j*128+p]
        et_h = np.ascontiguousarray(
            enc_i.reshape(BL, 2, T // P // 2, P, ENC).transpose(0, 1, 3, 2, 4)
        )
        ee_h = np.ascontiguousarray(
            enc_i.transpose(0, 2, 1)
            .reshape(BL, ENC // P, P, 2, T // 2)
            .transpose(0, 3, 2, 1, 4)
        )
        in_maps.append(
            {
                "enc_t": et_h,
                "enc_e": ee_h,
                "decT": np.ascontiguousarray(dec[sl].T),
                "Wa": wa,
                "Wb": wb,
                "Wv": wv,
                "bias2": bias2,
            }
        )
    return in_maps


def kernel(decoder_hidden, encoder_outputs, Wa, ba, Wb, bb, Wv, bv):
    from concourse.bass_utils import run_bass_kernel_spmd

    if "nc" not in _NC_CACHE:
        _NC_CACHE["nc"] = _build_nc()
    nc = _NC_CACHE["nc"]

    in_maps = build_in_maps(
        decoder_hidden, encoder_outputs, Wa, ba, Wb, bb, Wv, bv
    )
    res = run_bass_kernel_spmd(nc, in_maps, core_ids=list(range(NCORES))).results
    ctx = np.concatenate([res[i]["ctx"] for i in range(NCORES)], axis=0)
    attn = np.concatenate([res[i]["attn"] for i in range(NCORES)], axis=0)
    return ctx.astype(np.float32), attn.astype(np.float32)[..., None]
